# revision 38
# baseline (speedup 1.0000x reference)
"""Trainium2 Bass kernel for nn_Attention_74088185856351.

Strategy: data-parallel over batch (8 batches -> 8 NeuronCores), full
attention per core, everything bf16 on the PE.

Per-core pipeline (N=1024 tokens, C=768, H=12 heads, D=64):
  M1-T : qkT[cout, n] = wqkvT[cin, cout].T @ xT[cin, n]  -- q,k come out
         directly in [d, token] layout (no transposes anywhere)
  v-M1 : v[token, d] in natural layout (same inputs, swapped roles)
  stats: per-head sum / sum-of-squares via PE matmuls against per-tile
         block-diagonal ones stationaries, accumulated into one
         [56,1024] PSUM region (rows 32-aligned for engine access)
  norm : one batched finalize (var -> recip + ACT sqrt); q is only
         SCALED by rstd (k-hat is zero-mean so mu_q cancels in softmax);
         k centered+scaled.  Scales broadcast across the 64 d-partitions
         with selector-matrix matmuls, applied by vector mult.
  M2   : logitsT[nk, nq] per head (bf16, K=64), even head on PE rows
         0-63 / odd head on rows 64-127, interleaved
  exp  : ACT Exp with 1/8 scale folded, PSUM->SBUF bf16
  M3   : outT'[65, nq] = [v|1].T @ expT  -> row 64 = softmax denom S
  1/S  : four denominator rows per pair parked at 32-aligned slots of
         one [65,1024] tile -> ONE batched DVE reciprocal per pair,
         broadcast via K=1 ones matmul, applied one pair late so the
         PE never waits on the reciprocal chain
  proj : y = aoT.T @ wpT + bias
"""
import sys
sys.path.insert(0, '/opt/trn_rl_repo')
import numpy as np

B, N, C, H = 8, 1024, 768, 12
D = C // H          # 64
NP = N // 128       # 8 token chunks
KC = C // 128       # 6 contraction chunks
NPAIR = 6           # head pairs (2 heads per 128-row tile)

_CACHE = {}


def _build_nc():
    import concourse.bacc as bacc
    import concourse.tile as tile
    from concourse import mybir

    F32, BF16 = mybir.dt.float32, mybir.dt.bfloat16
    ALU, AF = mybir.AluOpType, mybir.ActivationFunctionType

    nc = bacc.Bacc("TRN2", target_bir_lowering=False, debug=False, num_devices=8)
    xT_d = nc.declare_dram_parameter("xT", [C, N], BF16, isOutput=False)
    wq_d = nc.declare_dram_parameter("wqkvT", [C, 3 * C], BF16, isOutput=False)
    wp_d = nc.declare_dram_parameter("wpT", [C, C], BF16, isOutput=False)
    bias_d = nc.declare_dram_parameter("bias", [1, C], BF16, isOutput=False)
    ones_d = nc.declare_dram_parameter("ones16", [128, 16], BF16, isOutput=False)
    eblk_d = nc.declare_dram_parameter("eblk", [128, 24 * 56], BF16, isOutput=False)
    wsum_d = nc.declare_dram_parameter("wsums", [C, 24], BF16, isOutput=False)
    sel_d = nc.declare_dram_parameter("sel", [24, 12 * 128], BF16, isOutput=False)
    selm_d = nc.declare_dram_parameter("selm", [24, 6 * 128], BF16, isOutput=False)
    y_d = nc.declare_dram_parameter("y", [N, C], F32, isOutput=True)

    with tile.TileContext(nc) as tc:
        with (
            tc.tile_pool(name="sbw", bufs=1) as sbw,
            tc.tile_pool(name="sba", bufs=1) as sba,
            tc.tile_pool(name="sbt", bufs=2) as sbt,
            tc.tile_pool(name="psb", bufs=2, space="PSUM") as psb,
            tc.tile_pool(name="pss", bufs=2, space="PSUM") as pss,
        ):
            # ---------------- loads ----------------
            ones_sb = sbw.tile([128, 16], BF16, tag="ones")
            eblk_sb = sbw.tile([128, 24 * 56], BF16, tag="eblk")
            sel_sb = sbw.tile([24, 12 * 128], BF16, tag="sel")
            selm_sb = sbw.tile([24, 6 * 128], BF16, tag="selm")
            xT = [sbw.tile([128, N], BF16, tag=f"xT{k}", name=f"xT{k}")
                  for k in range(KC)]
            wqk = [sbw.tile([128, 2 * C], BF16, tag=f"wqk{k}", name=f"wqk{k}")
                   for k in range(KC)]
            # x first on both queues -- the constants below aren't needed
            # until the first stats/apply matmuls
            for k in range(KC):
                eng = nc.sync if k % 2 == 0 else nc.gpsimd
                eng.dma_start(out=xT[k], in_=xT_d[k * 128:(k + 1) * 128, :])
            nc.gpsimd.dma_start(out=eblk_sb, in_=eblk_d[:, :])
            ws_sb = sbw.tile([128, KC * 24], BF16, tag="ws")
            for k in range(KC):
                nc.gpsimd.dma_start(out=ws_sb[:, k * 24:(k + 1) * 24],
                                    in_=wsum_d[k * 128:(k + 1) * 128, :])
            nc.gpsimd.dma_start(out=ones_sb, in_=ones_d[:, :])
            nc.gpsimd.dma_start(out=sel_sb, in_=sel_d[:, :])
            nc.gpsimd.dma_start(out=selm_sb, in_=selm_d[:, :])
            for cb in range(3):
                for k in range(KC):
                    nc.scalar.dma_start(
                        out=wqk[k][:, cb * 512:(cb + 1) * 512],
                        in_=wq_d[k * 128:(k + 1) * 128, cb * 512:(cb + 1) * 512])

            sbv_cm = tc.tile_pool(name="sbv", bufs=1)
            sbv = sbv_cm.__enter__()
            wv = [sbv.tile([128, C], BF16, tag=f"wv{k}", name=f"wv{k}")
                  for k in range(KC)]
            for k in range(KC):
                nc.gpsimd.dma_start(out=wv[k], in_=wq_d[k * 128:(k + 1) * 128, 2 * C:3 * C])
            wp = [sbw.tile([128, C], BF16, tag=f"wp{k}", name=f"wp{k}")
                  for k in range(KC)]
            for k in range(KC):
                nc.gpsimd.dma_start(out=wp[k], in_=wp_d[k * 128:(k + 1) * 128, :])
            import concourse.bass as bass
            bias_sb = sbw.tile([128, C], BF16, tag="bias")
            bias_bcast = bass.AP(tensor=bias_d.tensor if hasattr(bias_d, 'tensor') else bias_d,
                                 offset=0, ap=[[0, 128], [1, C]])
            nc.gpsimd.dma_start(out=bias_sb, in_=bias_bcast)

            # persistent activations
            qkraw = [sba.tile([128, N], BF16, tag=f"qkr{i}", name=f"qkr{i}")
                     for i in range(12)]
            qT = [sba.tile([128, N], BF16, tag=f"qT{p}", name=f"qT{p}")
                  for p in range(NPAIR)]
            kT = [sba.tile([128, N], BF16, tag=f"kT{p}", name=f"kT{p}")
                  for p in range(NPAIR)]
            v1 = [sba.tile([128, H, D + 1], BF16, tag=f"v1{n}", name=f"v1{n}")
                  for n in range(NP)]
            aoT = [sba.tile([128, N], BF16, tag=f"aoT{p}", name=f"aoT{p}")
                   for p in range(NPAIR)]
            rstd = sba.tile([24, N], BF16, tag="rstd")
            mrstd = sba.tile([24, N], BF16, tag="mrstd")
            # softmax denominators: 4 rows per pair at partitions 0/32/64/96
            # (engine partition bases must be 32-aligned); memset so the
            # unused partitions reciprocate safely
            # softmax denominators: 4 rows per pair parked at legal engine
            # bases: slots (0,L) (32,L) (64,L) (0,R) of a [65, 1024] tile;
            # memset so unused partitions reciprocate safely
            S4 = sba.tile([65, N], F32, tag="S4")
            rS4 = sba.tile([65, N], F32, tag="rS4")
            rS4bfs = [sba.tile([65, N], BF16, tag=f"rS4bf{i}", name=f"rS4bf{i}")
                      for i in range(2)]
            ones65 = sba.tile([65, 64], BF16, tag="ones65")
            nc.vector.memset(S4[:, :], 1.0)
            nc.vector.memset(ones65[:, :], 1.0)

            def _slot(j):
                # (partition base, column offset) for denominator slot j
                return (32 * j, 0) if j < 3 else (0, 512)

            ps_stat_cm = tc.tile_pool(name="psst", bufs=1, space="PSUM")
            ps_stat = ps_stat_cm.__enter__()
            stats = ps_stat.tile([56, N], F32, tag="stat")

            # ---------------- phase A: qk M1-T + stats ----------------
            # Each stat matmul uses a per-(tile, raw/sq) block-diagonal ones
            # stationary [128, 48] so the output lands at base partition 0
            # (rows other than 2i, 2i+1 get accumulated zeros).  All 48 MMs
            # per half form one long accumulation group over the stats tile.
            def emit_stats(i, sqt):
                for half in range(2):
                    nc.tensor.matmul(
                        stats[32:56, half * 512:(half + 1) * 512],
                        eblk_sb[:, (2 * i + 1) * 56 + 32:(2 * i + 1) * 56 + 56],
                        sqt[:, half * 512:(half + 1) * 512],
                        start=(i == 0), stop=(i == 11))

            def emit_raw_sums():
                # sum_d q = x . (sum_d W): one 12-MM accumulation against
                # host-precomputed per-head column sums of the qk weights
                for k in range(KC):
                    for half in range(2):
                        nc.tensor.matmul(
                            stats[0:24, half * 512:(half + 1) * 512],
                            ws_sb[:, k * 24:(k + 1) * 24],
                            xT[k][:, half * 512:(half + 1) * 512],
                            start=(k == 0), stop=(k == KC - 1))

            prev = None
            for i in range(12):
                pt = psb.tile([128, N], F32, tag="big", name="m1ps")
                for k in range(KC):
                    for half in range(2):
                        nc.tensor.matmul(
                            pt[:, half * 512:(half + 1) * 512],
                            wqk[k][:, i * 128:(i + 1) * 128],
                            xT[k][:, half * 512:(half + 1) * 512],
                            start=(k == 0), stop=(k == KC - 1))
                if prev is not None:
                    emit_stats(*prev)
                if i == 2:
                    emit_raw_sums()
                nc.vector.tensor_copy(qkraw[i], pt[:, :])
                sqt = sbt.tile([128, N], BF16, tag="sq", bufs=2)
                nc.vector.tensor_tensor(out=sqt, in0=qkraw[i], in1=qkraw[i],
                                        op=ALU.mult)
                prev = (i, sqt)
            emit_stats(*prev)

            # ---------------- phase B: v M1 (natural layout) ----------------
            def emit_v(n):
                pvs = [pss.tile([128, 384], F32, tag="small", name="vps")
                       for _ in range(2)]
                for k in range(KC):
                    for vh in range(2):
                        nc.tensor.matmul(
                            pvs[vh][:, :],
                            xT[k][:, n * 128:(n + 1) * 128],
                            wv[k][:, vh * 384:(vh + 1) * 384],
                            start=(k == 0), stop=(k == KC - 1))
                for vh, pv in enumerate(pvs):
                    nc.vector.tensor_copy(
                        v1[n][:, vh * 6:(vh + 1) * 6, 0:D],
                        pv[:, :].rearrange("p (g d) -> p g d", g=6))
                nc.vector.tensor_copy(
                    v1[n][:, :, D:D + 1].rearrange("p h one -> p (h one)"),
                    ones_sb[:, 0:H])

            emit_v(0)
            emit_v(1)
            emit_v(2)

            # ---------------- phase C: finalize stats ----------------
            # (emitted here so the serial ACT/vector chain overlaps the
            # v-M1 matmuls that keep the PE busy)
            # var63 = sumsq - sums^2/64  (unnormalized; 63/ folded into sqrt)
            t1 = sbt.tile([24, N], F32, tag="t1", bufs=1)
            nc.scalar.activation(out=t1, in_=stats[0:24, :], func=AF.Square)
            t2 = sbt.tile([24, N], F32, tag="t2", bufs=1)
            nc.vector.scalar_tensor_tensor(
                out=t2, in0=t1, scalar=-1.0 / D, in1=stats[32:56, :],
                op0=ALU.mult, op1=ALU.add)
            rvar = sbt.tile([24, N], F32, tag="rvar", bufs=1)
            nc.vector.reciprocal(out=rvar[:, 0:512], in_=t2[:, 0:512])
            emit_v(3)
            nc.vector.reciprocal(out=rvar[:, 512:N], in_=t2[:, 512:N])
            emit_v(4)
            # rstd = sqrt(63 / var63)
            nc.scalar.activation(out=rstd, in_=rvar, func=AF.Sqrt,
                                 scale=float(D - 1))
            # mrstd = -(mean * rstd) = sums * (-1/64) * rstd (all 24 rows;
            # the k selector later picks rows 12..23)
            nc.vector.scalar_tensor_tensor(
                out=mrstd, in0=stats[0:24, :], scalar=-1.0 / D, in1=rstd[0:24, :],
                op0=ALU.mult, op1=ALU.mult)

            for n in range(5, NP):
                emit_v(n)

            ps_stat_cm.__exit__(None, None, None)
            sbv_cm.__exit__(None, None, None)   # free wv region before et pool

            sbe_cm = tc.tile_pool(name="sbe", bufs=1)
            sbe = sbe_cm.__enter__()
            psb2_cm = tc.tile_pool(name="psb2", bufs=1, space="PSUM")
            psb2 = psb2_cm.__enter__()

            # ---------------- phase D: normalize + attention ----------------
            def apply_q(p):
                bq = psb2.tile([128, N], F32, tag="bc", name="bqps")
                for half in range(2):
                    nc.tensor.matmul(
                        bq[:, half * 512:(half + 1) * 512],
                        sel_sb[:, p * 128:(p + 1) * 128],
                        rstd[:, half * 512:(half + 1) * 512],
                        start=True, stop=True)
                nc.vector.tensor_tensor(out=qT[p], in0=qkraw[p], in1=bq, op=ALU.mult)

            def apply_k(p):
                br = psb2.tile([128, N], F32, tag="bc", name="brps")
                bm = psb2.tile([128, N], F32, tag="bc", name="bmps")
                for half in range(2):
                    nc.tensor.matmul(
                        br[:, half * 512:(half + 1) * 512],
                        sel_sb[:, (6 + p) * 128:(7 + p) * 128],
                        rstd[:, half * 512:(half + 1) * 512],
                        start=True, stop=True)
                for half in range(2):
                    nc.tensor.matmul(
                        bm[:, half * 512:(half + 1) * 512],
                        selm_sb[:, p * 128:(p + 1) * 128],
                        mrstd[:, half * 512:(half + 1) * 512],
                        start=True, stop=True)
                ktmp = sbt.tile([128, N], F32, tag="ktmp", bufs=2)
                nc.vector.tensor_tensor(out=ktmp, in0=qkraw[6 + p], in1=br, op=ALU.mult)
                nc.vector.tensor_tensor(out=kT[p], in0=ktmp, in1=bm, op=ALU.add)

            def attn_m2_pair(p, ets01):
                # even head on PE rows 0-63, odd head on rows 64-127 --
                # interleaved so row-disjoint matmuls can overlap on the
                # 32x32 sub-array grid
                for nk in range(NP):
                    p2s = [psb.tile([128, N], F32, tag="big", name="m2ps")
                           for _ in range(2)]
                    for qh in range(2):
                        for par in range(2):
                            ro = par * D
                            nc.tensor.matmul(
                                p2s[par][:, qh * 512:(qh + 1) * 512],
                                kT[p][ro:ro + D, nk * 128:(nk + 1) * 128],
                                qT[p][ro:ro + D, qh * 512:(qh + 1) * 512],
                                start=True, stop=True)
                    for par in range(2):
                        nc.scalar.activation(out=ets01[par][nk], in_=p2s[par],
                                             func=AF.Exp,
                                             scale=float(D) ** -0.5)

            def attn_m3_pair(p, ets01):
                # Both heads' four M3 halves, then ONE batched reciprocal
                # for the softmax denominators (DVE reciprocal cost scales
                # with free size only; rows parked at partitions 0/32/64/96
                # to satisfy engine base-alignment).
                p3sbs = []
                for par in range(2):
                    h = 2 * p + par
                    for qh in range(2):
                        j = 2 * par + qh
                        p3 = pss.tile([D + 1, 512], F32, tag="small", name="m3ps")
                        for nk in range(NP):
                            nc.tensor.matmul(
                                p3[:, :],
                                v1[nk][:, h, :],
                                ets01[par][nk][:, qh * 512:(qh + 1) * 512],
                                start=(nk == 0), stop=(nk == NP - 1))
                        # fast PSUM evac on scalar + S-row gather on vector
                        p3sb = sbt.tile([D, 512], BF16, tag=f"p3sb{j}", bufs=2)
                        nc.scalar.activation(out=p3sb, in_=p3[0:D, :],
                                             func=AF.Identity)
                        pb, co = _slot(j)
                        nc.vector.tensor_copy(S4[pb:pb + 1, co:co + 512],
                                              p3[D:D + 1, :])
                        p3sbs.append(p3sb)
                        if j == 2:
                            # slots 0-2 live in the left column block;
                            # reciprocate early so the tail can start
                            nc.vector.reciprocal(out=rS4[:, 0:512],
                                                 in_=S4[:, 0:512])
                            nc.scalar.activation(out=rS4bfs[p % 2][:, 0:512],
                                                 in_=rS4[:, 0:512],
                                                 func=AF.Identity)
                nc.vector.reciprocal(out=rS4[:, 512:N], in_=S4[:, 512:N])
                nc.scalar.activation(out=rS4bfs[p % 2][:, 512:N],
                                     in_=rS4[:, 512:N], func=AF.Identity)
                return p3sbs

            def attn_tail_pair(p, p3sbs):
                # 1/S broadcast across the 64 d-partitions via K=1 PE
                # matmul (emitted one pair late so the PE never waits on
                # the reciprocal chain)
                for par in range(2):
                    ro = par * D
                    for qh in range(2):
                        j = 2 * par + qh
                        pb, co = _slot(j)
                        bc = pss.tile([D, 512], F32, tag="small", name="bcps")
                        nc.tensor.matmul(
                            bc[:, :],
                            ones65[pb:pb + 1, :],
                            rS4bfs[p % 2][pb:pb + 1, co:co + 512],
                            start=True, stop=True)
                        nc.vector.tensor_tensor(
                            out=aoT[p][ro:ro + D, qh * 512:(qh + 1) * 512],
                            in0=p3sbs[j][:, :], in1=bc[:, :], op=ALU.mult)

            # applies are hoisted two pairs ahead so the qT/kT vector
            # mults enqueue before the previous pair's reciprocal chain
            apply_q(0); apply_k(0)
            apply_q(1); apply_k(1)
            pending = None
            for p in range(NPAIR):
                ets0 = [sbe.tile([128, N], BF16, tag=f"e0{nk}", name=f"e0{nk}")
                        for nk in range(NP)]
                ets1 = [sbe.tile([128, N], BF16, tag=f"e1{nk}", name=f"e1{nk}")
                        for nk in range(NP)]
                attn_m2_pair(p, (ets0, ets1))
                if p + 2 < NPAIR:
                    apply_q(p + 2)
                    apply_k(p + 2)
                prev_pending = pending
                pending = (p, attn_m3_pair(p, (ets0, ets1)))
                if prev_pending is not None:
                    attn_tail_pair(*prev_pending)
            attn_tail_pair(*pending)

            psb2_cm.__exit__(None, None, None)
            sbe_cm.__exit__(None, None, None)

            # ---------------- phase E: proj ----------------
            for n in range(NP):
                ysb = sbt.tile([128, C], F32, tag="y", bufs=2)
                pps = []
                for half in range(2):
                    pp = pss.tile([128, 384], F32, tag="small", name="pps")
                    for k in range(KC):
                        nc.tensor.matmul(
                            pp[:, :],
                            aoT[k][:, n * 128:(n + 1) * 128],
                            wp[k][:, half * 384:(half + 1) * 384],
                            start=(k == 0), stop=(k == KC - 1))
                    pps.append(pp)
                for half, pp in enumerate(pps):
                    nc.vector.tensor_tensor(
                        out=ysb[:, half * 384:(half + 1) * 384], in0=pp[:, :],
                        in1=bias_sb[:, half * 384:(half + 1) * 384], op=ALU.add)
                nc.scalar.dma_start(out=y_d[n * 128:(n + 1) * 128, :], in_=ysb[:, :])

    nc.compile()
    return nc


def _prep_inputs(x, qkv_w, proj_w, proj_b):
    import ml_dtypes
    bf16 = ml_dtypes.bfloat16
    wqkvT = np.ascontiguousarray(qkv_w.T).astype(bf16)          # [768, 2304]
    wsums = (wqkvT[:, :24 * 64].astype(np.float64)
             .reshape(C, 24, 64).sum(axis=2)).astype(bf16)       # [768, 24]
    wpT = np.ascontiguousarray(proj_w.T).astype(bf16)           # [768, 768]
    bias = proj_b.reshape(1, C).astype(bf16)
    ones16 = np.ones((128, 16), dtype=bf16)
    # eblk: 24 block-diagonal ones stationaries [128, 48]; slice j=2i+b
    # (b: 0=raw sums, 1=square sums) scatters tile i's two per-head column
    # sums to stats rows r0=24b+2i (head 2i, partitions 0:64) and r0+1.
    eblk = np.zeros((128, 24 * 56), dtype=np.float32)
    for i in range(12):
        for bbit in range(2):
            j = 2 * i + bbit
            r0 = 32 * bbit + 2 * i
            eblk[0:64, j * 56 + r0] = 1.0
            eblk[64:128, j * 56 + r0 + 1] = 1.0
    eblk = eblk.astype(bf16)
    # sel: per-pair selector [24, 128]; col m picks rstd row 2p (m<64)
    # or 2p+1 (m>=64); q pairs are slices 0-5, k pairs slices 6-11.
    sel = np.zeros((24, 12 * 128), dtype=np.float32)
    for j in range(12):
        r = 2 * (j % 6) + (12 if j >= 6 else 0)
        sel[r, j * 128:j * 128 + 64] = 1.0
        sel[r + 1, j * 128 + 64:(j + 1) * 128] = 1.0
    sel = sel.astype(bf16)
    selm = np.zeros((24, 6 * 128), dtype=np.float32)
    for j in range(6):
        selm[12 + 2 * j, j * 128:j * 128 + 64] = 1.0
        selm[13 + 2 * j, j * 128 + 64:(j + 1) * 128] = 1.0
    selm = selm.astype(bf16)
    maps = []
    for b in range(B):
        maps.append({
            "xT": np.ascontiguousarray(x[b].T).astype(bf16),
            "wqkvT": wqkvT, "wpT": wpT, "bias": bias,
            "ones16": ones16, "eblk": eblk, "sel": sel, "selm": selm,
            "wsums": wsums,
        })
    return maps


def kernel(x, qkv_w, proj_w, proj_b):
    from concourse.bass_utils import run_bass_kernel_spmd
    if "nc" not in _CACHE:
        _CACHE["nc"] = _build_nc()
    nc = _CACHE["nc"]
    maps = _prep_inputs(np.asarray(x), np.asarray(qkv_w),
                        np.asarray(proj_w), np.asarray(proj_b))
    import os
    trace = bool(os.environ.get("KERNEL_TRACE"))
    res = run_bass_kernel_spmd(nc, maps, list(range(B)), trace=trace)
    _CACHE["last_result"] = res
    out = np.stack([res.results[b]["y"] for b in range(B)], axis=0)
    return out.astype(np.float32)


# revision 40
# speedup vs baseline: 1.2248x; 1.2248x over previous
"""Trainium2 Bass kernel for nn_Attention_74088185856351.

Strategy: data-parallel over batch (8 batches -> 8 NeuronCores), full
attention per core, everything bf16 on the PE.

Per-core pipeline (N=1024 tokens, C=768, H=12 heads, D=64):
  M1-T : qkT[cout, n] = wqkvT[cin, cout].T @ xT[cin, n]  -- q,k come out
         directly in [d, token] layout (no transposes anywhere)
  v-M1 : v[token, d] in natural layout (same inputs, swapped roles)
  stats: per-head sum / sum-of-squares via PE matmuls against per-tile
         block-diagonal ones stationaries, accumulated into one
         [56,1024] PSUM region (rows 32-aligned for engine access)
  norm : one batched finalize (var -> recip + ACT sqrt); q is only
         SCALED by rstd (k-hat is zero-mean so mu_q cancels in softmax);
         k centered+scaled.  Scales broadcast across the 64 d-partitions
         with selector-matrix matmuls, applied by vector mult.
  M2   : logitsT[nk, nq] per head (bf16, K=64), even head on PE rows
         0-63 / odd head on rows 64-127, interleaved
  exp  : ACT Exp with 1/8 scale folded, PSUM->SBUF bf16
  M3   : outT'[65, nq] = [v|1].T @ expT  -> row 64 = softmax denom S
  1/S  : four denominator rows per pair parked at 32-aligned slots of
         one [65,1024] tile -> ONE batched DVE reciprocal per pair,
         broadcast via K=1 ones matmul, applied one pair late so the
         PE never waits on the reciprocal chain
  proj : y = aoT.T @ wpT + bias
"""
import sys
sys.path.insert(0, '/opt/trn_rl_repo')
import numpy as np

B, N, C, H = 8, 1024, 768, 12
D = C // H          # 64
NP = N // 128       # 8 token chunks
KC = C // 128       # 6 contraction chunks
NPAIR = 6           # head pairs (2 heads per 128-row tile)

_CACHE = {}


def _build_nc():
    import concourse.bacc as bacc
    import concourse.tile as tile
    from concourse import mybir

    F32, BF16 = mybir.dt.float32, mybir.dt.bfloat16
    ALU, AF = mybir.AluOpType, mybir.ActivationFunctionType

    nc = bacc.Bacc("TRN2", target_bir_lowering=False, debug=False, num_devices=8)
    xT_d = nc.declare_dram_parameter("xT", [C, N], BF16, isOutput=False)
    wq_d = nc.declare_dram_parameter("wqkvT", [C, 3 * C], BF16, isOutput=False)
    wp_d = nc.declare_dram_parameter("wpT", [C, C], BF16, isOutput=False)
    bias_d = nc.declare_dram_parameter("bias", [1, C], BF16, isOutput=False)
    ones_d = nc.declare_dram_parameter("ones16", [128, 16], BF16, isOutput=False)
    eblk_d = nc.declare_dram_parameter("eblk", [128, 24 * 56], BF16, isOutput=False)
    wsum_d = nc.declare_dram_parameter("wsums", [C, 24], BF16, isOutput=False)
    sel_d = nc.declare_dram_parameter("sel", [24, 12 * 128], BF16, isOutput=False)
    selm_d = nc.declare_dram_parameter("selm", [24, 6 * 128], BF16, isOutput=False)
    y_d = nc.declare_dram_parameter("y", [N, C], F32, isOutput=True)

    with tile.TileContext(nc) as tc:
        with (
            tc.tile_pool(name="sbw", bufs=1) as sbw,
            tc.tile_pool(name="sba", bufs=1) as sba,
            tc.tile_pool(name="sbt", bufs=2) as sbt,
            tc.tile_pool(name="psb", bufs=2, space="PSUM") as psb,
            tc.tile_pool(name="pss", bufs=2, space="PSUM") as pss,
        ):
            # ---------------- loads ----------------
            ones_sb = sbw.tile([128, 16], BF16, tag="ones")
            eblk_sb = sbw.tile([128, 24 * 56], BF16, tag="eblk")
            sel_sb = sbw.tile([24, 12 * 128], BF16, tag="sel")
            selm_sb = sbw.tile([24, 6 * 128], BF16, tag="selm")
            xT = [sbw.tile([128, N], BF16, tag=f"xT{k}", name=f"xT{k}")
                  for k in range(KC)]
            wqk = [sbw.tile([128, 2 * C], BF16, tag=f"wqk{k}", name=f"wqk{k}")
                   for k in range(KC)]
            # x first on both queues -- the constants below aren't needed
            # until the first stats/apply matmuls
            for k in range(KC):
                eng = nc.sync if k % 2 == 0 else nc.gpsimd
                eng.dma_start(out=xT[k], in_=xT_d[k * 128:(k + 1) * 128, :])
            nc.gpsimd.dma_start(out=eblk_sb, in_=eblk_d[:, :])
            ws_sb = sbw.tile([128, KC * 24], BF16, tag="ws")
            for k in range(KC):
                nc.gpsimd.dma_start(out=ws_sb[:, k * 24:(k + 1) * 24],
                                    in_=wsum_d[k * 128:(k + 1) * 128, :])
            nc.gpsimd.dma_start(out=ones_sb, in_=ones_d[:, :])
            nc.gpsimd.dma_start(out=sel_sb, in_=sel_d[:, :])
            nc.gpsimd.dma_start(out=selm_sb, in_=selm_d[:, :])
            for cb in range(3):
                for k in range(KC):
                    nc.scalar.dma_start(
                        out=wqk[k][:, cb * 512:(cb + 1) * 512],
                        in_=wq_d[k * 128:(k + 1) * 128, cb * 512:(cb + 1) * 512])

            sbv_cm = tc.tile_pool(name="sbv", bufs=1)
            sbv = sbv_cm.__enter__()
            wv = [sbv.tile([128, C], BF16, tag=f"wv{k}", name=f"wv{k}")
                  for k in range(KC)]
            for k in range(KC):
                nc.gpsimd.dma_start(out=wv[k], in_=wq_d[k * 128:(k + 1) * 128, 2 * C:3 * C])
            wp = [sbw.tile([128, C], BF16, tag=f"wp{k}", name=f"wp{k}")
                  for k in range(KC)]
            for k in range(KC):
                nc.gpsimd.dma_start(out=wp[k], in_=wp_d[k * 128:(k + 1) * 128, :])
            import concourse.bass as bass
            bias_sb = sbw.tile([128, C], BF16, tag="bias")
            bias_bcast = bass.AP(tensor=bias_d.tensor if hasattr(bias_d, 'tensor') else bias_d,
                                 offset=0, ap=[[0, 128], [1, C]])
            nc.gpsimd.dma_start(out=bias_sb, in_=bias_bcast)

            # persistent activations
            qkraw = [sba.tile([128, N], BF16, tag=f"qkr{i}", name=f"qkr{i}")
                     for i in range(12)]
            qT = [sba.tile([128, N], BF16, tag=f"qT{p}", name=f"qT{p}")
                  for p in range(NPAIR)]
            kT = [sba.tile([128, N], BF16, tag=f"kT{p}", name=f"kT{p}")
                  for p in range(NPAIR)]
            v1 = [sba.tile([128, H, D + 1], BF16, tag=f"v1{n}", name=f"v1{n}")
                  for n in range(NP)]
            aoT = [sba.tile([128, N], BF16, tag=f"aoT{p}", name=f"aoT{p}")
                   for p in range(NPAIR)]
            rstd = sba.tile([24, N], BF16, tag="rstd")
            mrstd = sba.tile([24, N], BF16, tag="mrstd")
            # softmax denominators: 4 rows per pair at partitions 0/32/64/96
            # (engine partition bases must be 32-aligned); memset so the
            # unused partitions reciprocate safely
            # softmax denominators: 4 rows per pair parked at legal engine
            # bases: slots (0,L) (32,L) (64,L) (0,R) of a [65, 1024] tile;
            # memset so unused partitions reciprocate safely
            S4 = sba.tile([65, N], F32, tag="S4")
            rS4 = sba.tile([65, N], F32, tag="rS4")
            rS4bfs = [sba.tile([65, N], BF16, tag=f"rS4bf{i}", name=f"rS4bf{i}")
                      for i in range(2)]
            ones65 = sba.tile([65, 64], BF16, tag="ones65")
            nc.vector.memset(S4[:, :], 1.0)
            nc.vector.memset(ones65[:, :], 1.0)

            def _slot(j):
                # (partition base, column offset) for denominator slot j
                return (32 * j, 0) if j < 3 else (0, 512)

            ps_stat_cm = tc.tile_pool(name="psst", bufs=1, space="PSUM")
            ps_stat = ps_stat_cm.__enter__()
            stats = ps_stat.tile([56, N], F32, tag="stat")

            # ---------------- phase A: qk M1-T + stats ----------------
            # Each stat matmul uses a per-(tile, raw/sq) block-diagonal ones
            # stationary [128, 48] so the output lands at base partition 0
            # (rows other than 2i, 2i+1 get accumulated zeros).  All 48 MMs
            # per half form one long accumulation group over the stats tile.
            def emit_stats(i, sqt):
                for half in range(2):
                    nc.tensor.matmul(
                        stats[32:56, half * 512:(half + 1) * 512],
                        eblk_sb[:, (2 * i + 1) * 56 + 32:(2 * i + 1) * 56 + 56],
                        sqt[:, half * 512:(half + 1) * 512],
                        start=(i == 0), stop=(i == 11))

            def emit_raw_sums():
                # sum_d q = x . (sum_d W): one 12-MM accumulation against
                # host-precomputed per-head column sums of the qk weights
                for k in range(KC):
                    for half in range(2):
                        nc.tensor.matmul(
                            stats[0:24, half * 512:(half + 1) * 512],
                            ws_sb[:, k * 24:(k + 1) * 24],
                            xT[k][:, half * 512:(half + 1) * 512],
                            start=(k == 0), stop=(k == KC - 1))

            prev = None
            for i in range(12):
                pt = psb.tile([128, N], F32, tag="big", name="m1ps")
                for k in range(KC):
                    for half in range(2):
                        nc.tensor.matmul(
                            pt[:, half * 512:(half + 1) * 512],
                            wqk[k][:, i * 128:(i + 1) * 128],
                            xT[k][:, half * 512:(half + 1) * 512],
                            start=(k == 0), stop=(k == KC - 1))
                if prev is not None:
                    emit_stats(*prev)
                if i == 2:
                    emit_raw_sums()
                nc.vector.tensor_copy(qkraw[i], pt[:, :])
                sqt = sbt.tile([128, N], BF16, tag="sq", bufs=2)
                nc.vector.tensor_tensor(out=sqt, in0=qkraw[i], in1=qkraw[i],
                                        op=ALU.mult)
                prev = (i, sqt)
            emit_stats(*prev)

            # ---------------- phase B: v M1 (natural layout) ----------------
            def emit_v(n):
                pvs = [pss.tile([128, 384], F32, tag="small", name="vps")
                       for _ in range(2)]
                for k in range(KC):
                    for vh in range(2):
                        nc.tensor.matmul(
                            pvs[vh][:, :],
                            xT[k][:, n * 128:(n + 1) * 128],
                            wv[k][:, vh * 384:(vh + 1) * 384],
                            start=(k == 0), stop=(k == KC - 1))
                for vh, pv in enumerate(pvs):
                    nc.vector.tensor_copy(
                        v1[n][:, vh * 6:(vh + 1) * 6, 0:D],
                        pv[:, :].rearrange("p (g d) -> p g d", g=6))
                nc.vector.tensor_copy(
                    v1[n][:, :, D:D + 1].rearrange("p h one -> p (h one)"),
                    ones_sb[:, 0:H])

            emit_v(0)
            emit_v(1)
            emit_v(2)

            # ---------------- phase C: finalize stats ----------------
            # (emitted here so the serial ACT/vector chain overlaps the
            # v-M1 matmuls that keep the PE busy)
            # var63 = sumsq - sums^2/64  (unnormalized; 63/ folded into sqrt)
            t1 = sbt.tile([24, N], F32, tag="t1", bufs=1)
            nc.scalar.activation(out=t1, in_=stats[0:24, :], func=AF.Square)
            t2 = sbt.tile([24, N], F32, tag="t2", bufs=1)
            nc.vector.scalar_tensor_tensor(
                out=t2, in0=t1, scalar=-1.0 / D, in1=stats[32:56, :],
                op0=ALU.mult, op1=ALU.add)
            emit_v(3)
            rvar = sbt.tile([24, N], F32, tag="rvar", bufs=1)
            nc.vector.reciprocal(out=rvar[:, 0:512], in_=t2[:, 0:512])
            emit_v(4)
            nc.vector.reciprocal(out=rvar[:, 512:N], in_=t2[:, 512:N])
            emit_v(5)
            # rstd = sqrt(63 / var63)
            nc.scalar.activation(out=rstd, in_=rvar, func=AF.Sqrt,
                                 scale=float(D - 1))
            # mrstd = -(mean * rstd) = sums * (-1/64) * rstd (all 24 rows;
            # the k selector later picks rows 12..23)
            nc.vector.scalar_tensor_tensor(
                out=mrstd, in0=stats[0:24, :], scalar=-1.0 / D, in1=rstd[0:24, :],
                op0=ALU.mult, op1=ALU.mult)

            for n in range(6, NP):
                emit_v(n)

            ps_stat_cm.__exit__(None, None, None)
            sbv_cm.__exit__(None, None, None)   # free wv region before et pool

            sbe_cm = tc.tile_pool(name="sbe", bufs=1)
            sbe = sbe_cm.__enter__()

            # ---------------- phase D: normalize + attention ----------------
            def apply_q(p):
                bq = psb.tile([128, N], F32, tag="big", name="bqps")
                for half in range(2):
                    nc.tensor.matmul(
                        bq[:, half * 512:(half + 1) * 512],
                        sel_sb[:, p * 128:(p + 1) * 128],
                        rstd[:, half * 512:(half + 1) * 512],
                        start=True, stop=True)
                nc.vector.tensor_tensor(out=qT[p], in0=qkraw[p], in1=bq, op=ALU.mult)

            def apply_k(p):
                br = psb.tile([128, N], F32, tag="big", name="brps")
                bm = psb.tile([128, N], F32, tag="big", name="bmps")
                for half in range(2):
                    nc.tensor.matmul(
                        br[:, half * 512:(half + 1) * 512],
                        sel_sb[:, (6 + p) * 128:(7 + p) * 128],
                        rstd[:, half * 512:(half + 1) * 512],
                        start=True, stop=True)
                for half in range(2):
                    nc.tensor.matmul(
                        bm[:, half * 512:(half + 1) * 512],
                        selm_sb[:, p * 128:(p + 1) * 128],
                        mrstd[:, half * 512:(half + 1) * 512],
                        start=True, stop=True)
                ktmp = sbt.tile([128, N], F32, tag="ktmp", bufs=2)
                nc.vector.tensor_tensor(out=ktmp, in0=qkraw[6 + p], in1=br, op=ALU.mult)
                nc.vector.tensor_tensor(out=kT[p], in0=ktmp, in1=bm, op=ALU.add)

            def attn_m2_pair(p, ets01):
                # even head on PE rows 0-63, odd head on rows 64-127 --
                # interleaved so row-disjoint matmuls can overlap on the
                # 32x32 sub-array grid
                for nk in range(NP):
                    p2s = [psb.tile([128, N], F32, tag="big", name="m2ps")
                           for _ in range(2)]
                    for qh in range(2):
                        for par in range(2):
                            ro = par * D
                            nc.tensor.matmul(
                                p2s[par][:, qh * 512:(qh + 1) * 512],
                                kT[p][ro:ro + D, nk * 128:(nk + 1) * 128],
                                qT[p][ro:ro + D, qh * 512:(qh + 1) * 512],
                                start=True, stop=True)
                    for par in range(2):
                        nc.scalar.activation(out=ets01[par][nk], in_=p2s[par],
                                             func=AF.Exp,
                                             scale=float(D) ** -0.5)

            def attn_m3_pair(p, ets01):
                # Both heads' four M3 halves, then ONE batched reciprocal
                # for the softmax denominators (DVE reciprocal cost scales
                # with free size only; rows parked at partitions 0/32/64/96
                # to satisfy engine base-alignment).
                p3sbs = []
                for par in range(2):
                    h = 2 * p + par
                    for qh in range(2):
                        j = 2 * par + qh
                        p3 = pss.tile([D + 1, 512], F32, tag="small", name="m3ps")
                        for nk in range(NP):
                            nc.tensor.matmul(
                                p3[:, :],
                                v1[nk][:, h, :],
                                ets01[par][nk][:, qh * 512:(qh + 1) * 512],
                                start=(nk == 0), stop=(nk == NP - 1))
                        # fast PSUM evac on scalar + S-row gather on vector
                        p3sb = sbt.tile([D, 512], BF16, tag=f"p3sb{j}", bufs=2)
                        nc.scalar.activation(out=p3sb, in_=p3[0:D, :],
                                             func=AF.Identity)
                        pb, co = _slot(j)
                        nc.vector.tensor_copy(S4[pb:pb + 1, co:co + 512],
                                              p3[D:D + 1, :])
                        p3sbs.append(p3sb)
                        if j == 2:
                            # slots 0-2 live in the left column block;
                            # reciprocate early so the tail can start
                            nc.vector.reciprocal(out=rS4[:, 0:512],
                                                 in_=S4[:, 0:512])
                            nc.scalar.activation(out=rS4bfs[p % 2][:, 0:512],
                                                 in_=rS4[:, 0:512],
                                                 func=AF.Identity)
                nc.vector.reciprocal(out=rS4[:, 512:N], in_=S4[:, 512:N])
                nc.scalar.activation(out=rS4bfs[p % 2][:, 512:N],
                                     in_=rS4[:, 512:N], func=AF.Identity)
                return p3sbs

            def attn_tail_pair(p, p3sbs):
                # 1/S broadcast across the 64 d-partitions via K=1 PE
                # matmul (emitted one pair late so the PE never waits on
                # the reciprocal chain)
                for par in range(2):
                    ro = par * D
                    for qh in range(2):
                        j = 2 * par + qh
                        pb, co = _slot(j)
                        bc = pss.tile([D, 512], F32, tag="small", name="bcps")
                        nc.tensor.matmul(
                            bc[:, :],
                            ones65[pb:pb + 1, :],
                            rS4bfs[p % 2][pb:pb + 1, co:co + 512],
                            start=True, stop=True)
                        nc.vector.tensor_tensor(
                            out=aoT[p][ro:ro + D, qh * 512:(qh + 1) * 512],
                            in0=p3sbs[j][:, :], in1=bc[:, :], op=ALU.mult)

            # applies are hoisted two pairs ahead so the qT/kT vector
            # mults enqueue before the previous pair's reciprocal chain
            apply_q(0); apply_k(0)
            apply_q(1); apply_k(1)
            pending = None
            for p in range(NPAIR):
                ets0 = [sbe.tile([128, N], BF16, tag=f"e0{nk}", name=f"e0{nk}")
                        for nk in range(NP)]
                ets1 = [sbe.tile([128, N], BF16, tag=f"e1{nk}", name=f"e1{nk}")
                        for nk in range(NP)]
                attn_m2_pair(p, (ets0, ets1))
                if p + 2 < NPAIR:
                    apply_q(p + 2)
                    apply_k(p + 2)
                prev_pending = pending
                pending = (p, attn_m3_pair(p, (ets0, ets1)))
                if prev_pending is not None:
                    attn_tail_pair(*prev_pending)
            attn_tail_pair(*pending)

            sbe_cm.__exit__(None, None, None)

            # ---------------- phase E: proj ----------------
            for n in range(NP):
                ysb = sbt.tile([128, C], F32, tag="y", bufs=2)
                pps = []
                for half in range(2):
                    pp = pss.tile([128, 384], F32, tag="small", name="pps")
                    for k in range(KC):
                        nc.tensor.matmul(
                            pp[:, :],
                            aoT[k][:, n * 128:(n + 1) * 128],
                            wp[k][:, half * 384:(half + 1) * 384],
                            start=(k == 0), stop=(k == KC - 1))
                    pps.append(pp)
                for half, pp in enumerate(pps):
                    nc.vector.tensor_tensor(
                        out=ysb[:, half * 384:(half + 1) * 384], in0=pp[:, :],
                        in1=bias_sb[:, half * 384:(half + 1) * 384], op=ALU.add)
                oeng = nc.scalar if n % 2 == 0 else nc.sync
                oeng.dma_start(out=y_d[n * 128:(n + 1) * 128, :], in_=ysb[:, :])

    nc.compile()
    return nc


def _prep_inputs(x, qkv_w, proj_w, proj_b):
    import ml_dtypes
    bf16 = ml_dtypes.bfloat16
    wqkvT = np.ascontiguousarray(qkv_w.T).astype(bf16)          # [768, 2304]
    wsums = (wqkvT[:, :24 * 64].astype(np.float64)
             .reshape(C, 24, 64).sum(axis=2)).astype(bf16)       # [768, 24]
    wpT = np.ascontiguousarray(proj_w.T).astype(bf16)           # [768, 768]
    bias = proj_b.reshape(1, C).astype(bf16)
    ones16 = np.ones((128, 16), dtype=bf16)
    # eblk: 24 block-diagonal ones stationaries [128, 48]; slice j=2i+b
    # (b: 0=raw sums, 1=square sums) scatters tile i's two per-head column
    # sums to stats rows r0=24b+2i (head 2i, partitions 0:64) and r0+1.
    eblk = np.zeros((128, 24 * 56), dtype=np.float32)
    for i in range(12):
        for bbit in range(2):
            j = 2 * i + bbit
            r0 = 32 * bbit + 2 * i
            eblk[0:64, j * 56 + r0] = 1.0
            eblk[64:128, j * 56 + r0 + 1] = 1.0
    eblk = eblk.astype(bf16)
    # sel: per-pair selector [24, 128]; col m picks rstd row 2p (m<64)
    # or 2p+1 (m>=64); q pairs are slices 0-5, k pairs slices 6-11.
    sel = np.zeros((24, 12 * 128), dtype=np.float32)
    for j in range(12):
        r = 2 * (j % 6) + (12 if j >= 6 else 0)
        sel[r, j * 128:j * 128 + 64] = 1.0
        sel[r + 1, j * 128 + 64:(j + 1) * 128] = 1.0
    sel = sel.astype(bf16)
    selm = np.zeros((24, 6 * 128), dtype=np.float32)
    for j in range(6):
        selm[12 + 2 * j, j * 128:j * 128 + 64] = 1.0
        selm[13 + 2 * j, j * 128 + 64:(j + 1) * 128] = 1.0
    selm = selm.astype(bf16)
    maps = []
    for b in range(B):
        maps.append({
            "xT": np.ascontiguousarray(x[b].T).astype(bf16),
            "wqkvT": wqkvT, "wpT": wpT, "bias": bias,
            "ones16": ones16, "eblk": eblk, "sel": sel, "selm": selm,
            "wsums": wsums,
        })
    return maps


def kernel(x, qkv_w, proj_w, proj_b):
    from concourse.bass_utils import run_bass_kernel_spmd
    if "nc" not in _CACHE:
        _CACHE["nc"] = _build_nc()
    nc = _CACHE["nc"]
    maps = _prep_inputs(np.asarray(x), np.asarray(qkv_w),
                        np.asarray(proj_w), np.asarray(proj_b))
    import os
    trace = bool(os.environ.get("KERNEL_TRACE"))
    res = run_bass_kernel_spmd(nc, maps, list(range(B)), trace=trace)
    _CACHE["last_result"] = res
    out = np.stack([res.results[b]["y"] for b in range(B)], axis=0)
    return out.astype(np.float32)
